# revision 12
# baseline (speedup 1.0000x reference)
"""Trainium2 Bass kernel for a time-of-flight Gaussian render module.

Math: for rays r (theta-major, R=1024), samples s (S=64), Gaussians g (G=2048):
    quad(s,r,g) = t_s^2 * (u_r . p_g) + t_s * (d_r . q_g) + C_g
with u_r the 6 symmetric products of the ray direction, p_g the packed
precision matrix, q_g = 2 P_g (o - mu_g), C_g = om^T P om - 2 ln(opac_g).
So quad = X @ Y with X [(s,r), 10] and Y [10, g]:  one tall matmul.
Then w = exp(-0.5 quad); density = sum_g w; rho = sum_g w * albedo_g;
transmittance via an exclusive cumsum over s (strictly-triangular matmul).

Sharding: rays split into 8 contiguous blocks of 128 (4 theta rows each);
Gaussian set replicated.  Each core computes its [64, 128] result block.

Device pipeline per core, per 1024-column block of (s,r) pairs:
  for each of 16 g-tiles (128 Gaussians):
    PE:  quad matmul, K=60 (3-way bf16 split of X/Y, 6 product pairs stacked
         on the contraction axis -- fp32 matmuls run at 1/4 rate, stacked
         bf16 pairs cost the same cycles as K=10 and keep ~24-bit accuracy)
    ACT: w = exp(-0.5 quad) -> fp16
    PE:  [density; rho] += [1; albedo]^T @ w  (accumulated in PSUM)
Host does only O(R*10 + G*10 + S*R) prep/assembly.
"""

import math
import numpy as np
import ml_dtypes

import concourse.bass as bass
import concourse.bacc as bacc
import concourse.mybir as mybir
from concourse.bass_utils import run_bass_kernel_spmd
from concourse.tile import TileContext

# ---- fixed problem constants (from the module definition) ----
G = 2048
NT, NP, S = 32, 32, 64
R = NT * NP                     # 1024 rays
N_CORES = 8
R_CORE = R // N_CORES           # 128 rays per core
SRC = S * R_CORE                # 8192 (s,r) columns per core
GT = G // 128                   # 16 gaussian tiles
SB = SRC // 1024                # 8 column blocks of 1024
THETA_RANGE = (0.0, math.pi / 2.0)
PHI_RANGE = (-math.pi / 2.0, math.pi / 2.0)
R_RANGE = (0.5, 2.5)
STEP = 1.0 * 0.03125            # C_LIGHT * DELTA_T
SH_C0 = 0.28209479177387814

F32 = mybir.dt.float32
BF16 = mybir.dt.bfloat16
FP16 = mybir.dt.float16

_NC_CACHE: dict = {}


def _split3(x):
    """float64 -> three bf16 planes whose sum reconstructs ~24 mantissa bits."""
    h = np.asarray(x, ml_dtypes.bfloat16)
    r1 = x - h.astype(np.float64)
    m = np.asarray(r1, ml_dtypes.bfloat16)
    r2 = r1 - m.astype(np.float64)
    l = np.asarray(r2, ml_dtypes.bfloat16)
    return h, m, l


def _build_nc(reps: int = 1, qp_bufs: int = 3, w_bufs: int = 4,
              rmm_lag: int = 2, xs_split: bool = True, act_pair: bool = False):
    key = (reps, qp_bufs, w_bufs, rmm_lag, xs_split, act_pair)
    if key in _NC_CACHE:
        return _NC_CACHE[key]
    nc = bacc.Bacc("TRN2", target_bir_lowering=False, debug=False,
                   num_devices=N_CORES)
    xstack_d = nc.dram_tensor("xstack", [60, SRC], BF16, kind="ExternalInput").ap()
    ystack_d = nc.dram_tensor("ystack", [60, G], BF16, kind="ExternalInput").ap()
    vt_d = nc.dram_tensor("vt", [128, 2 * GT], FP16, kind="ExternalInput").ap()
    m2_d = nc.dram_tensor("m2", [S, R_CORE], F32, kind="ExternalInput").ap()
    lt_d = nc.dram_tensor("lt", [S, S], F32, kind="ExternalInput").ap()
    res_d = nc.dram_tensor("res", [S, R_CORE], F32, kind="ExternalOutput").ap()

    with TileContext(nc) as tc:
        with (
            tc.tile_pool(name="const", bufs=1) as cpool,
            tc.tile_pool(name="w", bufs=w_bufs) as wpool,
            tc.tile_pool(name="stg", bufs=2) as spool,
            tc.tile_pool(name="fin", bufs=1) as fpool,
            tc.tile_pool(name="psum", bufs=1, space="PSUM") as ppool,
            tc.tile_pool(name="psq", bufs=(1 if act_pair else qp_bufs),
                         space="PSUM") as qpool,
        ):
            ys = cpool.tile([60, G], BF16, tag="ys")
            vt = cpool.tile([128, 2 * GT], FP16, tag="vt")
            m2t = cpool.tile([S, R_CORE], F32, tag="m2")
            ltt = cpool.tile([S, S], F32, tag="lt")
            # order matters: first compute needs ys + xs0 (+vt soon after);
            # each HWDGE trigger costs ~0.6us serially in front of the kernel
            nc.sync.dma_start(out=ys[:, 0:128], in_=ystack_d[:, 0:128])
            if xs_split:
                xs_tiles = []
                for sb in range(SB):
                    xt = cpool.tile([60, 1024], BF16, tag=f"xs{sb}")
                    xs_tiles.append(xt)
                nc.sync.dma_start(out=xs_tiles[0][:, :], in_=xstack_d[:, 0:1024])
                nc.sync.dma_start(out=ys[:, 128:G], in_=ystack_d[:, 128:G])
                nc.sync.dma_start(out=vt[:, :], in_=vt_d[:, :])
                for sb in range(1, SB):
                    nc.sync.dma_start(
                        out=xs_tiles[sb][:, :],
                        in_=xstack_d[:, sb * 1024:(sb + 1) * 1024])
            else:
                xs = cpool.tile([60, SRC], BF16, tag="xs")
                nc.sync.dma_start(out=xs[:, :], in_=xstack_d[:, :])
                nc.sync.dma_start(out=vt[:, :], in_=vt_d[:, :])
                xs_tiles = [xs[:, sb * 1024:(sb + 1) * 1024] for sb in range(SB)]
            nc.sync.dma_start(out=m2t[:, :], in_=m2_d[:, :])
            nc.sync.dma_start(out=ltt[:, :], in_=lt_d[:, :])

            qring = None
            if act_pair:
                # one 6-bank PSUM tile managed as a 3-slot ring so a single
                # exp activation can span two g-tiles (N=2048, half the
                # per-instruction overhead); Tile's bank-level overlap
                # tracking still serializes true same-bank reuse
                qring = qpool.tile([128, 3 * 1024], F32, tag="qring")
                qringv = qring[:, :].rearrange("p (u v) -> p u v", v=1024)

            for rep in range(reps):
                dens_d = nc.dram_tensor(f"densbuf{rep}", [2, SRC], F32).ap()
                dtrt = fpool.tile([S, 2 * R_CORE], F32, tag="dtrt")
                dt = dtrt[:, 0:R_CORE]
                rt = dtrt[:, R_CORE:2 * R_CORE]
                pr_count = 0
                for sb in range(SB):
                    c0 = sb * 1024
                    s0 = sb * 8
                    xsb = xs_tiles[sb]
                    acc0 = ppool.tile([2, 512], F32, tag="acc0")
                    acc1 = ppool.tile([2, 512], F32, tag="acc1")

                    def emit_qmm(g, dst):
                        ylhs = ys[:, g * 128:(g + 1) * 128]
                        nc.tensor.matmul(dst[:, 0:512], ylhs, xsb[:, 0:512],
                                         start=True, stop=True)
                        nc.tensor.matmul(dst[:, 512:1024], ylhs,
                                         xsb[:, 512:1024],
                                         start=True, stop=True)

                    def emit_rmm(g, wg):
                        vlhs = vt[:, 2 * g:2 * g + 2]
                        nc.tensor.matmul(acc0[:, :], vlhs, wg[:, 0:512],
                                         start=(g == 0), stop=(g == GT - 1))
                        nc.tensor.matmul(acc1[:, :], vlhs, wg[:, 512:1024],
                                         start=(g == 0), stop=(g == GT - 1))

                    if act_pair:
                        for pr in range(GT // 2):
                            g0, g1 = 2 * pr, 2 * pr + 1
                            sl0, sl1 = pr_count % 3, (pr_count + 1) % 3
                            pr_count += 2
                            emit_qmm(g0, qring[:, sl0 * 1024:(sl0 + 1) * 1024])
                            emit_qmm(g1, qring[:, sl1 * 1024:(sl1 + 1) * 1024])
                            w = wpool.tile([128, 2048], FP16, tag="w")
                            if sl1 == sl0 + 1:
                                nc.scalar.activation(
                                    w[:, :],
                                    qring[:, sl0 * 1024:(sl1 + 1) * 1024],
                                    mybir.ActivationFunctionType.Exp,
                                    scale=-0.5)
                                h0, h1 = 0, 1
                            else:      # wrap: slots (2,0) -> read {0,2}
                                nc.scalar.activation(
                                    w[:, :].rearrange("p (u v) -> p u v",
                                                      v=1024),
                                    qringv[:, 0:3:2, :],
                                    mybir.ActivationFunctionType.Exp,
                                    scale=-0.5)
                                h0, h1 = 1, 0   # half 0 holds slot0 = g1
                            emit_rmm(g0, w[:, h0 * 1024:(h0 + 1) * 1024])
                            emit_rmm(g1, w[:, h1 * 1024:(h1 + 1) * 1024])
                    else:
                        w_tiles = [None] * GT
                        for gt in range(GT):
                            qp = qpool.tile([128, 1024], F32, tag="qp")
                            emit_qmm(gt, qp)
                            w = wpool.tile([128, 1024], FP16, tag="w")
                            nc.scalar.activation(
                                w[:, :], qp[:, :],
                                mybir.ActivationFunctionType.Exp, scale=-0.5)
                            w_tiles[gt] = w
                            if gt >= rmm_lag:
                                emit_rmm(gt - rmm_lag, w_tiles[gt - rmm_lag])
                        for g in range(GT - rmm_lag, GT):
                            emit_rmm(g, w_tiles[g])
                    stg = spool.tile([2, 1024], F32, tag="stg")
                    nc.vector.tensor_copy(stg[:, 0:512], acc0[:, :])
                    nc.vector.tensor_copy(stg[:, 512:1024], acc1[:, :])
                    nc.sync.dma_start(out=dens_d[:, c0:c0 + 1024], in_=stg[:, :])
                    # pipelined re-layout: (s-major flat) -> [s, (d r)] tile
                    nc.sync.dma_start(
                        out=dtrt[s0:s0 + 8, :].rearrange(
                            "s (d r) -> s d r", d=2),
                        in_=dens_d[:, c0:c0 + 1024].rearrange(
                            "d (s r) -> s d r", r=R_CORE))

                # ---- per-ray transmittance scan + post-processing ----
                cp = ppool.tile([S, R_CORE], F32, tag="acc0")
                nc.tensor.matmul(cp[:, :], ltt[:, :], dt[:, :],
                                 start=True, stop=True)
                tt = fpool.tile([S, R_CORE], F32, tag="tt")
                nc.scalar.activation(tt[:, :], cp[:, :],
                                     mybir.ActivationFunctionType.Exp,
                                     scale=-STEP)
                rd = fpool.tile([S, R_CORE], F32, tag="rd")
                nc.vector.tensor_mul(rd[:, :], rt[:, :], tt[:, :])
                out_t = fpool.tile([S, R_CORE], F32, tag="out")
                nc.vector.tensor_mul(out_t[:, :], rd[:, :], m2t[:, :])
                nc.sync.dma_start(out=res_d[:, :], in_=out_t[:, :])
    nc.compile()
    _NC_CACHE[reps] = nc
    return nc


def _host_prep(gaussian_means, gaussian_scales, gaussian_rotations,
               gaussian_opacities, gaussian_features, camera_pos):
    """Build per-core input maps (all float64 math on host)."""
    means = np.asarray(gaussian_means, np.float64)
    scales = np.exp(np.asarray(gaussian_scales, np.float64))
    quats = np.asarray(gaussian_rotations, np.float64)
    opac_raw = np.asarray(gaussian_opacities, np.float64)[:, 0]
    feats = np.asarray(gaussian_features, np.float64)
    cam = np.asarray(camera_pos, np.float64)

    q = quats / (np.linalg.norm(quats, axis=-1, keepdims=True) + 1e-12)
    w0, x0, y0, z0 = q[:, 0], q[:, 1], q[:, 2], q[:, 3]
    Rm = np.stack([
        1 - 2 * (y0 * y0 + z0 * z0), 2 * (x0 * y0 - w0 * z0), 2 * (x0 * z0 + w0 * y0),
        2 * (x0 * y0 + w0 * z0), 1 - 2 * (x0 * x0 + z0 * z0), 2 * (y0 * z0 - w0 * x0),
        2 * (x0 * z0 - w0 * y0), 2 * (y0 * z0 + w0 * x0), 1 - 2 * (x0 * x0 + y0 * y0),
    ], axis=-1).reshape(-1, 3, 3)
    inv_s2 = 1.0 / (scales * scales)
    P = np.einsum('gij,gj,gkj->gik', Rm, inv_s2, Rm)
    opa = 1.0 / (1.0 + np.exp(-opac_raw))
    opa = np.clip(opa, 1e-300, 1.0)
    albedo = np.mean(np.clip(SH_C0 * feats + 0.5, 0.0, None), axis=-1)

    om = cam[None, :] - means                       # [G,3]
    p6 = np.stack([P[:, 0, 0], P[:, 1, 1], P[:, 2, 2],
                   2 * P[:, 0, 1], 2 * P[:, 0, 2], 2 * P[:, 1, 2]], axis=0)  # [6,G]
    q3 = 2.0 * np.einsum('gde,ge->dg', P, om)       # [3,G]
    Cq = np.einsum('gd,gde,ge->g', om, P, om)
    C1 = (Cq - 2.0 * np.log(opa))[None, :]          # [1,G]
    Y = np.concatenate([p6, q3, C1], axis=0)        # [10,G]

    theta = np.linspace(THETA_RANGE[0], THETA_RANGE[1], NT)
    phi = np.linspace(PHI_RANGE[0], PHI_RANGE[1], NP)
    tg, pg = np.meshgrid(theta, phi, indexing='ij')
    th, ph = tg.reshape(-1), pg.reshape(-1)
    dirs = np.stack([np.sin(th) * np.cos(ph),
                     np.sin(th) * np.sin(ph),
                     np.cos(th)], axis=1)           # [R,3]
    t = np.linspace(R_RANGE[0], R_RANGE[1], S)
    t2 = t * t
    dx, dy, dz = dirs[:, 0], dirs[:, 1], dirs[:, 2]
    u6 = np.stack([dx * dx, dy * dy, dz * dz,
                   dx * dy, dx * dz, dy * dz], axis=1)   # [R,6]

    # X[(s,r), 10] with s-major column order per core
    Xfull = np.empty((S, R, 10), np.float64)
    Xfull[:, :, 0:6] = t2[:, None, None] * u6[None, :, :]
    Xfull[:, :, 6:9] = t[:, None, None] * dirs[None, :, :]
    Xfull[:, :, 9] = 1.0

    Yh, Ym, Yl = _split3(Y)
    # product pairs: (h,h),(h,m),(h,l),(m,h),(m,m),(l,h)
    Ystack = np.concatenate([Yh, Ym, Yl, Yh, Ym, Yh], axis=0)   # [60,G]

    vt = np.zeros((128, 2 * GT), np.float16)
    for gt in range(GT):
        vt[:, 2 * gt] = 1.0
        vt[:, 2 * gt + 1] = albedo[gt * 128:(gt + 1) * 128].astype(np.float16)

    m2 = (np.sin(th)[None, :] / (t2[:, None] + 1e-8)).astype(np.float64)  # [S,R]
    lt = np.triu(np.ones((S, S), np.float32), k=1)   # lhsT of strict-lower L

    in_maps = []
    for c in range(N_CORES):
        rs = slice(c * R_CORE, (c + 1) * R_CORE)
        Xc = Xfull[:, rs, :].reshape(SRC, 10).T      # [10, SRC]
        Xh, Xm, Xl = _split3(Xc)
        Xstack = np.concatenate([Xh, Xh, Xh, Xm, Xm, Xl], axis=0)  # [60,SRC]
        in_maps.append({
            "xstack": Xstack,
            "ystack": Ystack,
            "vt": vt,
            "m2": m2[:, rs].astype(np.float32),
            "lt": lt,
        })
    return in_maps


def _run(inputs: dict, reps: int = 1):
    num_theta = int(inputs.get("num_theta", NT))
    num_phi = int(inputs.get("num_phi", NP))
    num_r = int(inputs.get("num_r", S))
    assert (num_theta, num_phi, num_r) == (NT, NP, S), \
        f"kernel hardcoded for (32,32,64), got {(num_theta, num_phi, num_r)}"
    in_maps = _host_prep(
        inputs["gaussian_means"], inputs["gaussian_scales"],
        inputs["gaussian_rotations"], inputs["gaussian_opacities"],
        inputs["gaussian_features"], inputs["camera_pos"])
    nc = _build_nc(reps)
    out = run_bass_kernel_spmd(nc, in_maps, list(range(N_CORES)))
    blocks = [out.results[c]["res"] for c in range(N_CORES)]   # [S, R_CORE] each
    result = np.concatenate(blocks, axis=1).reshape(S, NT, NP).astype(np.float32)
    dtheta = (THETA_RANGE[1] - THETA_RANGE[0]) / NT
    dphi = (PHI_RANGE[1] - PHI_RANGE[0]) / NP
    hist = (result.astype(np.float64).sum(axis=(1, 2)) * dtheta * dphi
            ).astype(np.float32)
    return result, hist


def kernel(**inputs):
    return _run(inputs, reps=1)


# revision 15
# speedup vs baseline: 1036.4322x; 1036.4322x over previous
"""Trainium2 Bass kernel for a time-of-flight Gaussian render module.

Math: for rays r (theta-major, R=1024), samples s (S=64), Gaussians g (G=2048):
    quad(s,r,g) = t_s^2 * (u_r . p_g) + t_s * (d_r . q_g) + C_g
with u_r the 6 symmetric products of the ray direction, p_g the packed
precision matrix, q_g = 2 P_g (o - mu_g), C_g = om^T P om - 2 ln(opac_g).
So quad = X @ Y with X [(s,r), 10] and Y [10, g]:  one tall matmul.
Then w = exp(-0.5 quad); density = sum_g w; rho = sum_g w * albedo_g;
transmittance via an exclusive cumsum over s (strictly-triangular matmul).

Sharding: rays split into 8 contiguous blocks of 128 (4 theta rows each);
Gaussian set replicated.  Each core computes its [64, 128] result block.

Device pipeline per core, per 1024-column block of (s,r) pairs:
  for each of 16 g-tiles (128 Gaussians):
    PE:  quad matmul, K=60 (3-way bf16 split of X/Y, 6 product pairs stacked
         on the contraction axis -- fp32 matmuls run at 1/4 rate, stacked
         bf16 pairs cost the same cycles as K=10 and keep ~24-bit accuracy)
    ACT: w = exp(-0.5 quad) -> fp16
    PE:  [density; rho] += [1; albedo]^T @ w  (accumulated in PSUM)
Host does only O(R*10 + G*10 + S*R) prep/assembly.
"""

import math
import numpy as np
import ml_dtypes

import concourse.bass as bass
import concourse.bacc as bacc
import concourse.mybir as mybir
from concourse.bass_utils import run_bass_kernel_spmd
from concourse.tile import TileContext

# ---- fixed problem constants (from the module definition) ----
G = 2048
NT, NP, S = 32, 32, 64
R = NT * NP                     # 1024 rays
N_CORES = 8
R_CORE = R // N_CORES           # 128 rays per core
SRC = S * R_CORE                # 8192 (s,r) columns per core
GT = G // 128                   # 16 gaussian tiles
SB = SRC // 1024                # 8 column blocks of 1024
THETA_RANGE = (0.0, math.pi / 2.0)
PHI_RANGE = (-math.pi / 2.0, math.pi / 2.0)
R_RANGE = (0.5, 2.5)
STEP = 1.0 * 0.03125            # C_LIGHT * DELTA_T
SH_C0 = 0.28209479177387814

F32 = mybir.dt.float32
BF16 = mybir.dt.bfloat16
FP16 = mybir.dt.float16

_NC_CACHE: dict = {}


def _split3(x):
    """float64 -> three bf16 planes whose sum reconstructs ~24 mantissa bits."""
    h = np.asarray(x, ml_dtypes.bfloat16)
    r1 = x - h.astype(np.float64)
    m = np.asarray(r1, ml_dtypes.bfloat16)
    r2 = r1 - m.astype(np.float64)
    l = np.asarray(r2, ml_dtypes.bfloat16)
    return h, m, l


def _build_nc(reps: int = 1, qp_bufs: int = 3, w_bufs: int = 4,
              rmm_lag: int = 2, xs_split: bool = True, act_pair: bool = False,
              blk1536: bool = True):
    key = (reps, qp_bufs, w_bufs, rmm_lag, xs_split, act_pair, blk1536)
    if key in _NC_CACHE:
        return _NC_CACHE[key]
    if blk1536:
        BLKS = [(i * 1536, 1536) for i in range(5)] + [(5 * 1536, 512)]
    else:
        BLKS = [(i * 1024, 1024) for i in range(SB)]
    nc = bacc.Bacc("TRN2", target_bir_lowering=False, debug=False,
                   num_devices=N_CORES)
    xstack_d = nc.dram_tensor("xstack", [60, SRC], BF16, kind="ExternalInput").ap()
    ystack_d = nc.dram_tensor("ystack", [60, G], BF16, kind="ExternalInput").ap()
    vt_d = nc.dram_tensor("vt", [128, 2 * GT], FP16, kind="ExternalInput").ap()
    m2_d = nc.dram_tensor("m2", [S, R_CORE], F32, kind="ExternalInput").ap()
    lt_d = nc.dram_tensor("lt", [S, S], F32, kind="ExternalInput").ap()
    res_d = nc.dram_tensor("res", [S, R_CORE], F32, kind="ExternalOutput").ap()

    with TileContext(nc) as tc:
        with (
            tc.tile_pool(name="const", bufs=1) as cpool,
            tc.tile_pool(name="w", bufs=w_bufs) as wpool,
            tc.tile_pool(name="stg", bufs=2) as spool,
            tc.tile_pool(name="fin", bufs=1) as fpool,
            tc.tile_pool(name="psum", bufs=1, space="PSUM") as ppool,
            tc.tile_pool(name="psq", bufs=(2 if blk1536 else qp_bufs),
                         space="PSUM") as qpool,
        ):
            ys = cpool.tile([60, G], BF16, tag="ys")
            vt = cpool.tile([128, 2 * GT], FP16, tag="vt")
            m2t = cpool.tile([S, R_CORE], F32, tag="m2")
            ltt = cpool.tile([S, S], F32, tag="lt")
            # order matters: first compute needs ys + xs0 (+vt soon after);
            # each HWDGE trigger costs ~0.6us serially in front of the kernel
            nc.sync.dma_start(out=ys[:, 0:128], in_=ystack_d[:, 0:128])
            xs_tiles = []
            for bi, (c0, width) in enumerate(BLKS):
                xt = cpool.tile([60, width], BF16, tag=f"xs{bi}")
                xs_tiles.append(xt)
            nc.sync.dma_start(out=xs_tiles[0][:, :],
                              in_=xstack_d[:, 0:BLKS[0][1]])
            nc.sync.dma_start(out=ys[:, 128:G], in_=ystack_d[:, 128:G])
            nc.sync.dma_start(out=vt[:, :], in_=vt_d[:, :])
            for bi, (c0, width) in list(enumerate(BLKS))[1:]:
                nc.sync.dma_start(out=xs_tiles[bi][:, :],
                                  in_=xstack_d[:, c0:c0 + width])
            nc.sync.dma_start(out=m2t[:, :], in_=m2_d[:, :])
            nc.sync.dma_start(out=ltt[:, :], in_=lt_d[:, :])

            for rep in range(reps):
                dens_d = nc.dram_tensor(f"densbuf{rep}", [2, SRC], F32).ap()
                dtrt = fpool.tile([S, 2 * R_CORE], F32, tag="dtrt")
                dt = dtrt[:, 0:R_CORE]
                rt = dtrt[:, R_CORE:2 * R_CORE]
                for bi, (c0, width) in enumerate(BLKS):
                    s0 = c0 // R_CORE
                    nch = width // 512
                    xsb = xs_tiles[bi]
                    if blk1536:
                        # accumulator chunks packed in ONE psum bank at
                        # partitions {0,32,64} via tile_position col-groups
                        # (has_written clearing is range-scoped: verified)
                        acc = ppool.tile([2 + 32 * (nch - 1), 512], F32,
                                         tag="acc")
                        acc_c = [acc[32 * c:32 * c + 2, :] for c in range(nch)]
                        tpos = [(0, 32 * c) if c else None for c in range(nch)]
                    else:
                        acc_c = [ppool.tile([2, 512], F32, tag=f"acc{c}")
                                 for c in range(nch)]
                        tpos = [None] * nch

                    def emit_rmm(g, wg):
                        vlhs = vt[:, 2 * g:2 * g + 2]
                        for c in range(nch):
                            nc.tensor.matmul(acc_c[c], vlhs,
                                             wg[:, 512 * c:512 * (c + 1)],
                                             start=(g == 0),
                                             stop=(g == GT - 1),
                                             tile_position=tpos[c])

                    w_tiles = [None] * GT
                    for gt in range(GT):
                        qp = qpool.tile([128, width], F32, tag="qp")
                        ylhs = ys[:, gt * 128:(gt + 1) * 128]
                        for c in range(nch):
                            nc.tensor.matmul(qp[:, 512 * c:512 * (c + 1)],
                                             ylhs,
                                             xsb[:, 512 * c:512 * (c + 1)],
                                             start=True, stop=True)
                        w = wpool.tile([128, width], FP16, tag="w")
                        nc.scalar.activation(w[:, :], qp[:, :],
                                             mybir.ActivationFunctionType.Exp,
                                             scale=-0.5)
                        w_tiles[gt] = w
                        if gt >= rmm_lag:
                            emit_rmm(gt - rmm_lag, w_tiles[gt - rmm_lag])
                    for g in range(GT - rmm_lag, GT):
                        emit_rmm(g, w_tiles[g])
                    stg = spool.tile([2, width], F32, tag="stg")
                    for c in range(nch):
                        nc.vector.tensor_copy(stg[:, 512 * c:512 * (c + 1)],
                                              acc_c[c])
                    nc.sync.dma_start(out=dens_d[:, c0:c0 + width],
                                      in_=stg[:, :])
                    # pipelined re-layout: (s-major flat) -> [s, (d r)] tile
                    nc.sync.dma_start(
                        out=dtrt[s0:s0 + width // R_CORE, :].rearrange(
                            "s (d r) -> s d r", d=2),
                        in_=dens_d[:, c0:c0 + width].rearrange(
                            "d (s r) -> s d r", r=R_CORE))

                # ---- per-ray transmittance scan + post-processing ----
                cp = ppool.tile([S, R_CORE], F32, tag="acc" if blk1536 else "acc0")
                nc.tensor.matmul(cp[:, :], ltt[:, :], dt[:, :],
                                 start=True, stop=True)
                tt = fpool.tile([S, R_CORE], F32, tag="tt")
                nc.scalar.activation(tt[:, :], cp[:, :],
                                     mybir.ActivationFunctionType.Exp,
                                     scale=-STEP)
                rd = fpool.tile([S, R_CORE], F32, tag="rd")
                nc.vector.tensor_mul(rd[:, :], rt[:, :], tt[:, :])
                out_t = fpool.tile([S, R_CORE], F32, tag="out")
                nc.vector.tensor_mul(out_t[:, :], rd[:, :], m2t[:, :])
                nc.sync.dma_start(out=res_d[:, :], in_=out_t[:, :])
    nc.compile()
    _NC_CACHE[key] = nc
    return nc


def _host_prep(gaussian_means, gaussian_scales, gaussian_rotations,
               gaussian_opacities, gaussian_features, camera_pos):
    """Build per-core input maps (all float64 math on host)."""
    means = np.asarray(gaussian_means, np.float64)
    scales = np.exp(np.asarray(gaussian_scales, np.float64))
    quats = np.asarray(gaussian_rotations, np.float64)
    opac_raw = np.asarray(gaussian_opacities, np.float64)[:, 0]
    feats = np.asarray(gaussian_features, np.float64)
    cam = np.asarray(camera_pos, np.float64)

    q = quats / (np.linalg.norm(quats, axis=-1, keepdims=True) + 1e-12)
    w0, x0, y0, z0 = q[:, 0], q[:, 1], q[:, 2], q[:, 3]
    Rm = np.stack([
        1 - 2 * (y0 * y0 + z0 * z0), 2 * (x0 * y0 - w0 * z0), 2 * (x0 * z0 + w0 * y0),
        2 * (x0 * y0 + w0 * z0), 1 - 2 * (x0 * x0 + z0 * z0), 2 * (y0 * z0 - w0 * x0),
        2 * (x0 * z0 - w0 * y0), 2 * (y0 * z0 + w0 * x0), 1 - 2 * (x0 * x0 + y0 * y0),
    ], axis=-1).reshape(-1, 3, 3)
    inv_s2 = 1.0 / (scales * scales)
    P = np.einsum('gij,gj,gkj->gik', Rm, inv_s2, Rm)
    opa = 1.0 / (1.0 + np.exp(-opac_raw))
    opa = np.clip(opa, 1e-300, 1.0)
    albedo = np.mean(np.clip(SH_C0 * feats + 0.5, 0.0, None), axis=-1)

    om = cam[None, :] - means                       # [G,3]
    p6 = np.stack([P[:, 0, 0], P[:, 1, 1], P[:, 2, 2],
                   2 * P[:, 0, 1], 2 * P[:, 0, 2], 2 * P[:, 1, 2]], axis=0)  # [6,G]
    q3 = 2.0 * np.einsum('gde,ge->dg', P, om)       # [3,G]
    Cq = np.einsum('gd,gde,ge->g', om, P, om)
    C1 = (Cq - 2.0 * np.log(opa))[None, :]          # [1,G]
    Y = np.concatenate([p6, q3, C1], axis=0)        # [10,G]

    theta = np.linspace(THETA_RANGE[0], THETA_RANGE[1], NT)
    phi = np.linspace(PHI_RANGE[0], PHI_RANGE[1], NP)
    tg, pg = np.meshgrid(theta, phi, indexing='ij')
    th, ph = tg.reshape(-1), pg.reshape(-1)
    dirs = np.stack([np.sin(th) * np.cos(ph),
                     np.sin(th) * np.sin(ph),
                     np.cos(th)], axis=1)           # [R,3]
    t = np.linspace(R_RANGE[0], R_RANGE[1], S)
    t2 = t * t
    dx, dy, dz = dirs[:, 0], dirs[:, 1], dirs[:, 2]
    u6 = np.stack([dx * dx, dy * dy, dz * dz,
                   dx * dy, dx * dz, dy * dz], axis=1)   # [R,6]

    # X[(s,r), 10] with s-major column order per core
    Xfull = np.empty((S, R, 10), np.float64)
    Xfull[:, :, 0:6] = t2[:, None, None] * u6[None, :, :]
    Xfull[:, :, 6:9] = t[:, None, None] * dirs[None, :, :]
    Xfull[:, :, 9] = 1.0

    Yh, Ym, Yl = _split3(Y)
    # product pairs: (h,h),(h,m),(h,l),(m,h),(m,m),(l,h)
    Ystack = np.concatenate([Yh, Ym, Yl, Yh, Ym, Yh], axis=0)   # [60,G]

    vt = np.zeros((128, 2 * GT), np.float16)
    for gt in range(GT):
        vt[:, 2 * gt] = 1.0
        vt[:, 2 * gt + 1] = albedo[gt * 128:(gt + 1) * 128].astype(np.float16)

    m2 = (np.sin(th)[None, :] / (t2[:, None] + 1e-8)).astype(np.float64)  # [S,R]
    lt = np.triu(np.ones((S, S), np.float32), k=1)   # lhsT of strict-lower L

    in_maps = []
    for c in range(N_CORES):
        rs = slice(c * R_CORE, (c + 1) * R_CORE)
        Xc = Xfull[:, rs, :].reshape(SRC, 10).T      # [10, SRC]
        Xh, Xm, Xl = _split3(Xc)
        Xstack = np.concatenate([Xh, Xh, Xh, Xm, Xm, Xl], axis=0)  # [60,SRC]
        in_maps.append({
            "xstack": Xstack,
            "ystack": Ystack,
            "vt": vt,
            "m2": m2[:, rs].astype(np.float32),
            "lt": lt,
        })
    return in_maps


def _run(inputs: dict, reps: int = 1):
    num_theta = int(inputs.get("num_theta", NT))
    num_phi = int(inputs.get("num_phi", NP))
    num_r = int(inputs.get("num_r", S))
    assert (num_theta, num_phi, num_r) == (NT, NP, S), \
        f"kernel hardcoded for (32,32,64), got {(num_theta, num_phi, num_r)}"
    in_maps = _host_prep(
        inputs["gaussian_means"], inputs["gaussian_scales"],
        inputs["gaussian_rotations"], inputs["gaussian_opacities"],
        inputs["gaussian_features"], inputs["camera_pos"])
    nc = _build_nc(reps)
    out = run_bass_kernel_spmd(nc, in_maps, list(range(N_CORES)))
    blocks = [out.results[c]["res"] for c in range(N_CORES)]   # [S, R_CORE] each
    result = np.concatenate(blocks, axis=1).reshape(S, NT, NP).astype(np.float32)
    dtheta = (THETA_RANGE[1] - THETA_RANGE[0]) / NT
    dphi = (PHI_RANGE[1] - PHI_RANGE[0]) / NP
    hist = (result.astype(np.float64).sum(axis=(1, 2)) * dtheta * dphi
            ).astype(np.float32)
    return result, hist


def kernel(**inputs):
    return _run(inputs, reps=1)


# revision 18
# speedup vs baseline: 1047.2596x; 1.0104x over previous
"""Trainium2 Bass kernel for a time-of-flight Gaussian render module.

Math: for rays r (theta-major, R=1024), samples s (S=64), Gaussians g (G=2048):
    quad(s,r,g) = t_s^2 * (u_r . p_g) + t_s * (d_r . q_g) + C_g
with u_r the 6 symmetric products of the ray direction, p_g the packed
precision matrix, q_g = 2 P_g (o - mu_g), C_g = om^T P om - 2 ln(opac_g).
So quad = X @ Y with X [(s,r), 10] and Y [10, g]:  one tall matmul.
Then w = exp(-0.5 quad); density = sum_g w; rho = sum_g w * albedo_g;
transmittance via an exclusive cumsum over s (strictly-triangular matmul).

Sharding: rays split into 8 contiguous blocks of 128 (4 theta rows each);
Gaussian set replicated.  Each core computes its [64, 128] result block.

Device pipeline per core, per column block (5x1536 + 1x512 (s,r) columns):
  for each of 16 g-tiles (128 Gaussians):
    PE:  quad matmul, K=60 (3-way bf16 split of X/Y, 6 product pairs stacked
         on the contraction axis -- fp32 matmuls run at 1/4 rate, stacked
         bf16 pairs cost the same cycles as K=10 and keep ~24-bit accuracy)
    ACT: w = exp(-0.5 quad) -> fp16   (the ~1.2GHz*128-lane ScalarE is the
         throughput floor: ~109us/core of exp work)
    PE:  [density; rho] += [1; albedo]^T @ w, accumulated over g-tiles in
         one PSUM bank: the 3 column-chunks live at partitions {0,32,64}
         via tile_position col-groups (has_written clears are range-scoped)
Host does only O(R*10 + G*10 + S*R) prep/assembly.
"""

import math
import numpy as np
import ml_dtypes

import concourse.bass as bass
import concourse.bacc as bacc
import concourse.mybir as mybir
from concourse.bass_utils import run_bass_kernel_spmd
from concourse.tile import TileContext

# ---- fixed problem constants (from the module definition) ----
G = 2048
NT, NP, S = 32, 32, 64
R = NT * NP                     # 1024 rays
N_CORES = 8
R_CORE = R // N_CORES           # 128 rays per core
SRC = S * R_CORE                # 8192 (s,r) columns per core
GT = G // 128                   # 16 gaussian tiles
SB = SRC // 1024                # 8 column blocks of 1024
THETA_RANGE = (0.0, math.pi / 2.0)
PHI_RANGE = (-math.pi / 2.0, math.pi / 2.0)
R_RANGE = (0.5, 2.5)
STEP = 1.0 * 0.03125            # C_LIGHT * DELTA_T
SH_C0 = 0.28209479177387814

F32 = mybir.dt.float32
BF16 = mybir.dt.bfloat16
FP16 = mybir.dt.float16

_NC_CACHE: dict = {}


def _split3(x):
    """float64 -> three bf16 planes whose sum reconstructs ~24 mantissa bits."""
    h = np.asarray(x, ml_dtypes.bfloat16)
    r1 = x - h.astype(np.float64)
    m = np.asarray(r1, ml_dtypes.bfloat16)
    r2 = r1 - m.astype(np.float64)
    l = np.asarray(r2, ml_dtypes.bfloat16)
    return h, m, l


def _build_nc(reps: int = 1, qp_bufs: int = 3, w_bufs: int = 4,
              rmm_lag: int = 2, xs_split: bool = True, act_pair: bool = False,
              blk1536: bool = True):
    key = (reps, qp_bufs, w_bufs, rmm_lag, xs_split, act_pair, blk1536)
    if key in _NC_CACHE:
        return _NC_CACHE[key]
    if blk1536:
        BLKS = [(i * 1536, 1536) for i in range(5)] + [(5 * 1536, 512)]
    else:
        BLKS = [(i * 1024, 1024) for i in range(SB)]
    nc = bacc.Bacc("TRN2", target_bir_lowering=False, debug=False,
                   num_devices=N_CORES)
    xstack_d = nc.dram_tensor("xstack", [60, SRC], BF16, kind="ExternalInput").ap()
    ystack_d = nc.dram_tensor("ystack", [60, G], BF16, kind="ExternalInput").ap()
    vt_d = nc.dram_tensor("vt", [128, 2 * GT], FP16, kind="ExternalInput").ap()
    m2_d = nc.dram_tensor("m2", [S, R_CORE], F32, kind="ExternalInput").ap()
    lt_d = nc.dram_tensor("lt", [S, S], F32, kind="ExternalInput").ap()
    res_d = nc.dram_tensor("res", [S, R_CORE], F32, kind="ExternalOutput").ap()

    with TileContext(nc) as tc:
        with (
            tc.tile_pool(name="const", bufs=1) as cpool,
            tc.tile_pool(name="w", bufs=w_bufs) as wpool,
            tc.tile_pool(name="stg", bufs=2) as spool,
            tc.tile_pool(name="fin", bufs=1) as fpool,
            tc.tile_pool(name="psum", bufs=1, space="PSUM") as ppool,
            tc.tile_pool(name="psq", bufs=(2 if blk1536 else qp_bufs),
                         space="PSUM") as qpool,
        ):
            ys = cpool.tile([60, G], BF16, tag="ys")
            vt = cpool.tile([128, 2 * GT], FP16, tag="vt")
            m2t = cpool.tile([S, R_CORE], F32, tag="m2")
            ltt = cpool.tile([S, S], F32, tag="lt")
            # order matters: first compute needs ys + xs0 (+vt soon after);
            # each HWDGE trigger costs ~0.6us serially in front of the kernel
            nc.sync.dma_start(out=ys[:, 0:128], in_=ystack_d[:, 0:128])
            xs_tiles = []
            for bi, (c0, width) in enumerate(BLKS):
                xt = cpool.tile([60, width], BF16, tag=f"xs{bi}")
                xs_tiles.append(xt)
            nc.sync.dma_start(out=xs_tiles[0][:, :],
                              in_=xstack_d[:, 0:BLKS[0][1]])
            nc.sync.dma_start(out=ys[:, 128:G], in_=ystack_d[:, 128:G])
            nc.sync.dma_start(out=vt[:, :], in_=vt_d[:, :])
            for bi, (c0, width) in list(enumerate(BLKS))[1:]:
                nc.sync.dma_start(out=xs_tiles[bi][:, :],
                                  in_=xstack_d[:, c0:c0 + width])
            nc.sync.dma_start(out=m2t[:, :], in_=m2_d[:, :])
            nc.sync.dma_start(out=ltt[:, :], in_=lt_d[:, :])

            for rep in range(reps):
                dens_d = nc.dram_tensor(f"densbuf{rep}", [2, SRC], F32).ap()
                dtrt = fpool.tile([S, 2 * R_CORE], F32, tag="dtrt")
                dt = dtrt[:, 0:R_CORE]
                rt = dtrt[:, R_CORE:2 * R_CORE]
                for bi, (c0, width) in enumerate(BLKS):
                    s0 = c0 // R_CORE
                    nch = width // 512
                    xsb = xs_tiles[bi]
                    if blk1536:
                        # accumulator chunks packed in ONE psum bank at
                        # partitions {0,32,64} via tile_position col-groups
                        # (has_written clearing is range-scoped: verified)
                        acc = ppool.tile([2 + 32 * (nch - 1), 512], F32,
                                         tag="acc")
                        acc_c = [acc[32 * c:32 * c + 2, :] for c in range(nch)]
                        tpos = [(0, 32 * c) if c else None for c in range(nch)]
                    else:
                        acc_c = [ppool.tile([2, 512], F32, tag=f"acc{c}")
                                 for c in range(nch)]
                        tpos = [None] * nch

                    def emit_rmm(g, wg):
                        vlhs = vt[:, 2 * g:2 * g + 2]
                        for c in range(nch):
                            nc.tensor.matmul(acc_c[c], vlhs,
                                             wg[:, 512 * c:512 * (c + 1)],
                                             start=(g == 0),
                                             stop=(g == GT - 1),
                                             tile_position=tpos[c])

                    w_tiles = [None] * GT
                    for gt in range(GT):
                        qp = qpool.tile([128, width], F32, tag="qp")
                        ylhs = ys[:, gt * 128:(gt + 1) * 128]
                        for c in range(nch):
                            nc.tensor.matmul(qp[:, 512 * c:512 * (c + 1)],
                                             ylhs,
                                             xsb[:, 512 * c:512 * (c + 1)],
                                             start=True, stop=True)
                        w = wpool.tile([128, width], FP16, tag="w")
                        nc.scalar.activation(w[:, :], qp[:, :],
                                             mybir.ActivationFunctionType.Exp,
                                             scale=-0.5)
                        w_tiles[gt] = w
                        if gt >= rmm_lag:
                            emit_rmm(gt - rmm_lag, w_tiles[gt - rmm_lag])
                    for g in range(GT - rmm_lag, GT):
                        emit_rmm(g, w_tiles[g])
                    stg = spool.tile([2, width], F32, tag="stg")
                    for c in range(nch):
                        nc.vector.tensor_copy(stg[:, 512 * c:512 * (c + 1)],
                                              acc_c[c])
                    nc.sync.dma_start(out=dens_d[:, c0:c0 + width],
                                      in_=stg[:, :])
                    # pipelined re-layout: (s-major flat) -> [s, (d r)] tile
                    nc.sync.dma_start(
                        out=dtrt[s0:s0 + width // R_CORE, :].rearrange(
                            "s (d r) -> s d r", d=2),
                        in_=dens_d[:, c0:c0 + width].rearrange(
                            "d (s r) -> s d r", r=R_CORE))

                # ---- per-ray transmittance scan + post-processing ----
                cp = ppool.tile([S, R_CORE], F32, tag="acc" if blk1536 else "acc0")
                nc.tensor.matmul(cp[:, :], ltt[:, :], dt[:, :],
                                 start=True, stop=True)
                tt = fpool.tile([S, R_CORE], F32, tag="tt")
                nc.scalar.activation(tt[:, :], cp[:, :],
                                     mybir.ActivationFunctionType.Exp,
                                     scale=-STEP)
                rd = fpool.tile([S, R_CORE], F32, tag="rd")
                nc.vector.tensor_mul(rd[:, :], rt[:, :], tt[:, :])
                out_t = fpool.tile([S, R_CORE], F32, tag="out")
                nc.vector.tensor_mul(out_t[:, :], rd[:, :], m2t[:, :])
                nc.sync.dma_start(out=res_d[:, :], in_=out_t[:, :])
    nc.compile()
    _NC_CACHE[key] = nc
    return nc


def _host_prep(gaussian_means, gaussian_scales, gaussian_rotations,
               gaussian_opacities, gaussian_features, camera_pos):
    """Build per-core input maps (all float64 math on host)."""
    means = np.asarray(gaussian_means, np.float64)
    scales = np.exp(np.asarray(gaussian_scales, np.float64))
    quats = np.asarray(gaussian_rotations, np.float64)
    opac_raw = np.asarray(gaussian_opacities, np.float64)[:, 0]
    feats = np.asarray(gaussian_features, np.float64)
    cam = np.asarray(camera_pos, np.float64)

    q = quats / (np.linalg.norm(quats, axis=-1, keepdims=True) + 1e-12)
    w0, x0, y0, z0 = q[:, 0], q[:, 1], q[:, 2], q[:, 3]
    Rm = np.stack([
        1 - 2 * (y0 * y0 + z0 * z0), 2 * (x0 * y0 - w0 * z0), 2 * (x0 * z0 + w0 * y0),
        2 * (x0 * y0 + w0 * z0), 1 - 2 * (x0 * x0 + z0 * z0), 2 * (y0 * z0 - w0 * x0),
        2 * (x0 * z0 - w0 * y0), 2 * (y0 * z0 + w0 * x0), 1 - 2 * (x0 * x0 + y0 * y0),
    ], axis=-1).reshape(-1, 3, 3)
    inv_s2 = 1.0 / (scales * scales)
    P = np.einsum('gij,gj,gkj->gik', Rm, inv_s2, Rm)
    opa = 1.0 / (1.0 + np.exp(-opac_raw))
    opa = np.clip(opa, 1e-300, 1.0)
    albedo = np.mean(np.clip(SH_C0 * feats + 0.5, 0.0, None), axis=-1)

    om = cam[None, :] - means                       # [G,3]
    p6 = np.stack([P[:, 0, 0], P[:, 1, 1], P[:, 2, 2],
                   2 * P[:, 0, 1], 2 * P[:, 0, 2], 2 * P[:, 1, 2]], axis=0)  # [6,G]
    q3 = 2.0 * np.einsum('gde,ge->dg', P, om)       # [3,G]
    Cq = np.einsum('gd,gde,ge->g', om, P, om)
    C1 = (Cq - 2.0 * np.log(opa))[None, :]          # [1,G]
    Y = np.concatenate([p6, q3, C1], axis=0)        # [10,G]

    theta = np.linspace(THETA_RANGE[0], THETA_RANGE[1], NT)
    phi = np.linspace(PHI_RANGE[0], PHI_RANGE[1], NP)
    tg, pg = np.meshgrid(theta, phi, indexing='ij')
    th, ph = tg.reshape(-1), pg.reshape(-1)
    dirs = np.stack([np.sin(th) * np.cos(ph),
                     np.sin(th) * np.sin(ph),
                     np.cos(th)], axis=1)           # [R,3]
    t = np.linspace(R_RANGE[0], R_RANGE[1], S)
    t2 = t * t
    dx, dy, dz = dirs[:, 0], dirs[:, 1], dirs[:, 2]
    u6 = np.stack([dx * dx, dy * dy, dz * dz,
                   dx * dy, dx * dz, dy * dz], axis=1)   # [R,6]

    # X[(s,r), 10] with s-major column order per core
    Xfull = np.empty((S, R, 10), np.float64)
    Xfull[:, :, 0:6] = t2[:, None, None] * u6[None, :, :]
    Xfull[:, :, 6:9] = t[:, None, None] * dirs[None, :, :]
    Xfull[:, :, 9] = 1.0

    Yh, Ym, Yl = _split3(Y)
    # product pairs: (h,h),(h,m),(h,l),(m,h),(m,m),(l,h)
    Ystack = np.concatenate([Yh, Ym, Yl, Yh, Ym, Yh], axis=0)   # [60,G]

    vt = np.zeros((128, 2 * GT), np.float16)
    for gt in range(GT):
        vt[:, 2 * gt] = 1.0
        vt[:, 2 * gt + 1] = albedo[gt * 128:(gt + 1) * 128].astype(np.float16)

    m2 = (np.sin(th)[None, :] / (t2[:, None] + 1e-8)).astype(np.float64)  # [S,R]
    lt = np.triu(np.ones((S, S), np.float32), k=1)   # lhsT of strict-lower L

    in_maps = []
    for c in range(N_CORES):
        rs = slice(c * R_CORE, (c + 1) * R_CORE)
        Xc = Xfull[:, rs, :].reshape(SRC, 10).T      # [10, SRC]
        Xh, Xm, Xl = _split3(Xc)
        Xstack = np.concatenate([Xh, Xh, Xh, Xm, Xm, Xl], axis=0)  # [60,SRC]
        in_maps.append({
            "xstack": Xstack,
            "ystack": Ystack,
            "vt": vt,
            "m2": m2[:, rs].astype(np.float32),
            "lt": lt,
        })
    return in_maps


def _run(inputs: dict, reps: int = 1):
    num_theta = int(inputs.get("num_theta", NT))
    num_phi = int(inputs.get("num_phi", NP))
    num_r = int(inputs.get("num_r", S))
    assert (num_theta, num_phi, num_r) == (NT, NP, S), \
        f"kernel hardcoded for (32,32,64), got {(num_theta, num_phi, num_r)}"
    in_maps = _host_prep(
        inputs["gaussian_means"], inputs["gaussian_scales"],
        inputs["gaussian_rotations"], inputs["gaussian_opacities"],
        inputs["gaussian_features"], inputs["camera_pos"])
    nc = _build_nc(reps)
    out = run_bass_kernel_spmd(nc, in_maps, list(range(N_CORES)))
    blocks = [out.results[c]["res"] for c in range(N_CORES)]   # [S, R_CORE] each
    result = np.concatenate(blocks, axis=1).reshape(S, NT, NP).astype(np.float32)
    dtheta = (THETA_RANGE[1] - THETA_RANGE[0]) / NT
    dphi = (PHI_RANGE[1] - PHI_RANGE[0]) / NP
    hist = (result.astype(np.float64).sum(axis=(1, 2)) * dtheta * dphi
            ).astype(np.float32)
    return result, hist


def kernel(**inputs):
    return _run(inputs, reps=1)
